# revision 6
# baseline (speedup 1.0000x reference)
"""Trainium2 Bass kernel for nn_MemoryReader.

Reference computation (per batch b):
    mi = mk.reshape(CK, N);  qi = qk.reshape(CK, P) / sqrt(CK)
    S  = mi.T @ qi                      # [N, P] affinity logits
    A  = softmax(S, axis=0)             # over memory axis N
    mem = mv.reshape(CV, N) @ A         # [CV, P]
    out = concat([mem, qv], axis=channel)

Sharding: 8 cores = (4 batches) x (2 halves of the memory axis N).
Each core computes, for its (b, half):
    E      = exp(S_half)                        # no max subtraction (logits ~ N(0,1))
    mem_un = mv_half @ E                        # [CV, P] unnormalized numerator
    lsum   = ones @ E                           # [1, P] denominator part
The host combines: mem = (mem_un_0 + mem_un_1) / (lsum_0 + lsum_1), then
concats qv (pure passthrough). No on-device collectives needed.

Device layout notes:
  - E is produced directly in [n(partition), p(free)] layout by computing
    S = mk_tile.T @ qk (lhsT = mk slice, K=CK=64 on partitions).
  - The second matmul contracts over n, so both operands need n on
    partitions: mv is pre-transposed ON THE HOST into [128, NT, CV]
    (partition-major tiles), making the device program transpose-free.
  - 1/sqrt(CK) is folded into the exp activation's free affine scale.
"""

import numpy as np
import ml_dtypes

import concourse.bass as bass
import concourse.tile as tile
from concourse import bacc, mybir
from concourse.bass_utils import run_bass_kernel_spmd

# Problem shape (hardcoded per contract)
B, CK, CV, T, H, W = 4, 64, 512, 8, 30, 54
N = T * H * W          # 12960 memory positions
P = H * W              # 1620 query positions
NHALF = N // 2         # 6480 per core
NT = (NHALF + 127) // 128   # 51 n-tiles (last has 80 rows)
NLAST = NHALF - (NT - 1) * 128  # 80
NPAD = NT * 128        # 6528
PCH = 405              # p-chunk (405*4B = 1620B <= one 2KB PSUM bank)
NPC = P // PCH         # 4 chunks

# Matmul precision mode: "bf16" (1 cyc/col), "f32r" (fp32 data, ~1 cyc/col
# at free>=256), "f32" (4 cyc/col).
MM_MODE = "bf16"

_CACHE = {}


def _mm_dtype():
    return mybir.dt.bfloat16 if MM_MODE == "bf16" else mybir.dt.float32


def _np_dtype():
    return ml_dtypes.bfloat16 if MM_MODE == "bf16" else np.float32


def _mm_ap(ap):
    """Operand view handed to the tensor engine."""
    if MM_MODE == "f32r":
        return ap.bitcast(mybir.dt.float32r)
    return ap


def _build_program():
    dt = _mm_dtype()
    f32 = mybir.dt.float32
    # Bacc (not plain Bass): its compile() runs generate_event_semaphores,
    # which splits multi-wait sync_info onto EventSemaphore instructions
    # (TRN2 allows only one wait per regular instruction).
    nc = bacc.Bacc(None, target_bir_lowering=False, debug=False)

    mk_d = nc.declare_dram_parameter("mk", [CK, NHALF], dt, isOutput=False)
    qk_d = nc.declare_dram_parameter("qk", [CK, P], dt, isOutput=False)
    mvt_d = nc.declare_dram_parameter("mvT", [128, NT, CV], dt, isOutput=False)
    mem_d = nc.declare_dram_parameter("mem", [CV, P], f32, isOutput=True)
    l_d = nc.declare_dram_parameter("lsum", [1, P], f32, isOutput=True)

    with tile.TileContext(nc) as tc:
        with (
            tc.tile_pool(name="singles", bufs=1) as singles,
            tc.tile_pool(name="epool", bufs=3) as epool,
            tc.tile_pool(name="opool", bufs=8) as opool,
            tc.tile_pool(name="olpool", bufs=2) as olpool,
            tc.tile_pool(name="spsum", bufs=2, space="PSUM") as spsum,
            tc.tile_pool(name="accpsum", bufs=4, space="PSUM") as accpsum,
            tc.tile_pool(name="lpsum", bufs=1, space="PSUM") as lpsum,
        ):
            mk_sb = singles.tile([CK, NHALF], dt)
            nc.sync.dma_start(out=mk_sb, in_=mk_d[:])
            qk_sb = singles.tile([CK, P], dt)
            nc.sync.dma_start(out=qk_sb, in_=qk_d[:])
            mvt_sb = singles.tile([128, NT, CV], dt)
            # split the load along the n-tile axis: each weight-tile read then
            # depends on exactly one DMA (avoids multi-sem wait explosion),
            # and the chunks land in consumption order across parallel queues
            NTG = 3
            for g in range(0, NT, NTG):
                g1 = min(g + NTG, NT)
                nc.sync.dma_start(
                    out=mvt_sb[:, g:g1, :],
                    in_=mvt_d[:, g:g1, :],
                )
            ones_sb = singles.tile([128, 1], dt)
            nc.vector.memset(ones_sb, 1.0)

            def mm1(ps, nt, s_tile):
                nsz = 128 if nt < NT - 1 else NLAST
                nc.tensor.matmul(
                    s_tile[:nsz],
                    lhsT=_mm_ap(mk_sb[:, nt * 128:nt * 128 + nsz]),
                    rhs=_mm_ap(qk_sb[:, ps:ps + PCH]),
                    start=True,
                    stop=True,
                )

            for pc in range(NPC):
                ps = pc * PCH
                acc = []
                for vt in range(4):
                    acc.append(accpsum.tile([128, PCH], f32, tag="acc", name="acc"))
                lacc = lpsum.tile([1, PCH], f32, tag="lacc", name="lacc")

                s_cur = spsum.tile([128, PCH], f32, tag="s", name="s")
                mm1(ps, 0, s_cur)
                for nt in range(NT):
                    nsz = 128 if nt < NT - 1 else NLAST
                    # issue next S ahead of this iteration's PV matmuls so
                    # the exp on ACT overlaps PE work
                    if nt + 1 < NT:
                        s_nxt = spsum.tile([128, PCH], f32, tag="s", name="s")
                        mm1(ps, nt + 1, s_nxt)
                    e_sb = epool.tile([128, PCH], dt, tag="e", name="e")
                    nc.scalar.activation(
                        out=e_sb[:nsz],
                        in_=s_cur[:nsz],
                        func=mybir.ActivationFunctionType.Exp,
                        scale=0.125,  # 1/sqrt(CK)
                    )
                    first, last = nt == 0, nt == NT - 1
                    for vt in range(4):
                        nc.tensor.matmul(
                            acc[vt],
                            lhsT=_mm_ap(mvt_sb[:nsz, nt, vt * 128:(vt + 1) * 128]),
                            rhs=_mm_ap(e_sb[:nsz]),
                            start=first,
                            stop=last,
                        )
                    nc.tensor.matmul(
                        lacc,
                        lhsT=_mm_ap(ones_sb[:nsz]),
                        rhs=_mm_ap(e_sb[:nsz]),
                        start=first,
                        stop=last,
                    )
                    if nt + 1 < NT:
                        s_cur = s_nxt

                for vt in range(4):
                    o_sb = opool.tile([128, PCH], f32, tag="o", name="o")
                    nc.vector.tensor_copy(out=o_sb, in_=acc[vt])
                    nc.sync.dma_start(
                        out=mem_d[vt * 128:(vt + 1) * 128, ps:ps + PCH], in_=o_sb
                    )
                ol_sb = olpool.tile([1, PCH], f32, tag="ol", name="ol")
                nc.vector.tensor_copy(out=ol_sb, in_=lacc)
                nc.sync.dma_start(out=l_d[:, ps:ps + PCH], in_=ol_sb)

    nc.compile()
    return nc


def _get_program():
    if "nc" not in _CACHE:
        _CACHE["nc"] = _build_program()
    return _CACHE["nc"]


def _make_in_maps(mk, mv, qk):
    npdt = _np_dtype()
    mkf = np.ascontiguousarray(mk.reshape(B, CK, N))
    mvf = np.ascontiguousarray(mv.reshape(B, CV, N))
    qkf = np.ascontiguousarray(qk.reshape(B, CK, P))
    in_maps = []
    for core in range(8):
        b, half = core // 2, core % 2
        n0, n1 = half * NHALF, (half + 1) * NHALF
        mk_c = np.ascontiguousarray(mkf[b, :, n0:n1]).astype(npdt)
        qk_c = qkf[b].astype(npdt)
        mvt = np.zeros((NPAD, CV), dtype=npdt)
        mvt[:NHALF] = mvf[b, :, n0:n1].T
        # partition-major tiles: [128, NT, CV], elem (p, t, v) = mvT[t*128+p, v]
        mvt_c = np.ascontiguousarray(mvt.reshape(NT, 128, CV).transpose(1, 0, 2))
        in_maps.append({"mk": mk_c, "qk": qk_c, "mvT": mvt_c})
    return in_maps


def _run(mk, mv, qk, qv, trace=False, **spmd_kwargs):
    nc = _get_program()
    in_maps = _make_in_maps(mk, mv, qk)
    res = run_bass_kernel_spmd(nc, in_maps, list(range(8)), trace=trace, **spmd_kwargs)
    out = np.empty((B, 2 * CV, P), dtype=np.float32)
    for b in range(B):
        m0, l0 = res.results[2 * b]["mem"], res.results[2 * b]["lsum"]
        m1, l1 = res.results[2 * b + 1]["mem"], res.results[2 * b + 1]["lsum"]
        out[b, :CV] = (m0 + m1) / (l0 + l1)
        out[b, CV:] = qv[b].reshape(CV, P)
    return out.reshape(B, 2 * CV, H, W), res


def kernel(mk, mv, qk, qv):
    out, _ = _run(np.asarray(mk), np.asarray(mv), np.asarray(qk), np.asarray(qv))
    return out


# revision 9
# speedup vs baseline: 1.1083x; 1.1083x over previous
"""Trainium2 Bass kernel for nn_MemoryReader.

Reference computation (per batch b):
    mi = mk.reshape(CK, N);  qi = qk.reshape(CK, P) / sqrt(CK)
    S  = mi.T @ qi                      # [N, P] affinity logits
    A  = softmax(S, axis=0)             # over memory axis N
    mem = mv.reshape(CV, N) @ A         # [CV, P]
    out = concat([mem, qv], axis=channel)

Sharding: 8 cores = (4 batches) x (2 halves of the memory axis N).
Each core computes, for its (b, half):
    E      = exp(S_half)                        # no max subtraction (logits ~ N(0,1))
    mem_un = mv_half @ E                        # [CV, P] unnormalized numerator
    lsum   = ones @ E                           # [1, P] denominator part
The host combines: mem = (mem_un_0 + mem_un_1) / (lsum_0 + lsum_1), then
concats qv (pure passthrough). No on-device collectives needed.

Device layout notes:
  - E is produced directly in [n(partition), p(free)] layout by computing
    S = mk_tile.T @ qk (lhsT = mk slice, K=CK=64 on partitions).
  - The second matmul contracts over n, so both operands need n on
    partitions: mv is pre-transposed ON THE HOST into [128, NT, CV]
    (partition-major tiles), making the device program transpose-free.
  - 1/sqrt(CK) is folded into the exp activation's free affine scale.
"""

import numpy as np
import ml_dtypes

import concourse.bass as bass
import concourse.tile as tile
from concourse import bacc, mybir
from concourse.bass_utils import run_bass_kernel_spmd

# Problem shape (hardcoded per contract)
B, CK, CV, T, H, W = 4, 64, 512, 8, 30, 54
N = T * H * W          # 12960 memory positions
P = H * W              # 1620 query positions
NHALF = N // 2         # 6480 per core
NT = (NHALF + 127) // 128   # 51 n-tiles (last has 80 rows)
NLAST = NHALF - (NT - 1) * 128  # 80
NPAD = NT * 128        # 6528
NPAIR = NT // 2        # 25 row-packed mm1 pairs (+1 solo leftover tile)
PCH = 405              # p-chunk (405*4B = 1620B <= one 2KB PSUM bank)
NPC = P // PCH         # 4 chunks

# Matmul precision mode: "bf16" (1 cyc/col), "f32r" (fp32 data, ~1 cyc/col
# at free>=256), "f32" (4 cyc/col).
MM_MODE = "bf16"

_CACHE = {}


def _mm_dtype():
    return mybir.dt.bfloat16 if MM_MODE == "bf16" else mybir.dt.float32


def _np_dtype():
    return ml_dtypes.bfloat16 if MM_MODE == "bf16" else np.float32


def _mm_ap(ap):
    """Operand view handed to the tensor engine."""
    if MM_MODE == "f32r":
        return ap.bitcast(mybir.dt.float32r)
    return ap


def _build_program():
    dt = _mm_dtype()
    f32 = mybir.dt.float32
    # Bacc (not plain Bass): its compile() runs generate_event_semaphores,
    # which splits multi-wait sync_info onto EventSemaphore instructions
    # (TRN2 allows only one wait per regular instruction).
    nc = bacc.Bacc(None, target_bir_lowering=False, debug=False)

    # mk2: row-pair-packed mk. Pair j holds n-tile 2j in partitions 0:64 and
    # n-tile 2j+1 in partitions 64:128 (the PE contraction dim is CK=64, so
    # two mm1 matmuls run concurrently in disjoint row groups). qk2 is qk
    # duplicated into partitions 64:128 (the row-group-1 matmul streams its
    # rhs from those partitions).
    mk2_d = nc.declare_dram_parameter("mk2", [128, NPAIR + 1, 128], dt, isOutput=False)
    qk2_d = nc.declare_dram_parameter("qk2", [128, P], dt, isOutput=False)
    mvt_d = nc.declare_dram_parameter("mvT", [128, NT, CV], dt, isOutput=False)
    mem_d = nc.declare_dram_parameter("mem", [CV, P], f32, isOutput=True)
    l_d = nc.declare_dram_parameter("lsum", [1, P], f32, isOutput=True)

    with tile.TileContext(nc) as tc:
        with (
            tc.tile_pool(name="singles", bufs=1) as singles,
            tc.tile_pool(name="epool", bufs=3) as epool,
            tc.tile_pool(name="opool", bufs=8) as opool,
            tc.tile_pool(name="olpool", bufs=2) as olpool,
            tc.tile_pool(name="spsum", bufs=3, space="PSUM") as spsum,
            tc.tile_pool(name="accpsum", bufs=4, space="PSUM") as accpsum,
            tc.tile_pool(name="lpsum", bufs=1, space="PSUM") as lpsum,
        ):
            ones_sb = singles.tile([128, 1], dt)
            nc.vector.memset(ones_sb, 1.0)
            qk2_sb = singles.tile([128, P], dt)
            nc.sync.dma_start(out=qk2_sb, in_=qk2_d[:])
            mk2_sb = singles.tile([128, NPAIR + 1, 128], dt)
            for g in range(0, NPAIR + 1, 7):
                g1 = min(g + 7, NPAIR + 1)
                nc.sync.dma_start(out=mk2_sb[:, g:g1, :], in_=mk2_d[:, g:g1, :])
            mvt_sb = singles.tile([128, NT, CV], dt)
            # split the load along the n-tile axis: each weight-tile read then
            # depends on exactly one DMA (avoids multi-sem wait explosion),
            # and the chunks land in consumption order across parallel queues
            NTG = 3
            for g in range(0, NT, NTG):
                g1 = min(g + NTG, NT)
                nc.sync.dma_start(
                    out=mvt_sb[:, g:g1, :],
                    in_=mvt_d[:, g:g1, :],
                )

            # Warm-up: ~128 tiny matmuls depending only on the ones memset.
            # They run while the input DMAs land, filling the initial PE idle
            # gap and releasing the HAM clock throttle (~3.4us of activity)
            # before the real work starts.
            warm = lpsum.tile([1, PCH], f32, tag="lacc", name="warm")
            for _ in range(128):
                nc.tensor.matmul(
                    warm[0:1, 0:1],
                    lhsT=_mm_ap(ones_sb),
                    rhs=_mm_ap(ones_sb),
                    start=True,
                    stop=True,
                )

            # n-tile groups: 25 row-packed pairs + the 80-row leftover tile
            groups = [(2 * j, 2 * j + 1) for j in range(NPAIR)] + [(NT - 1,)]

            def issue_group(ps, g, s_pool_tiles):
                if len(groups[g]) == 2:
                    s_a = spsum.tile([128, PCH], f32, tag="s", name="s")
                    s_b = spsum.tile([128, PCH], f32, tag="s", name="s")
                    nc.tensor.matmul(
                        s_a,
                        lhsT=_mm_ap(mk2_sb[0:64, g, :]),
                        rhs=_mm_ap(qk2_sb[0:64, ps:ps + PCH]),
                        start=True,
                        stop=True,
                        tile_position=(0, 0),
                    )
                    nc.tensor.matmul(
                        s_b,
                        lhsT=_mm_ap(mk2_sb[64:128, g, :]),
                        rhs=_mm_ap(qk2_sb[64:128, ps:ps + PCH]),
                        start=True,
                        stop=True,
                        tile_position=(64, 0),
                    )
                    s_pool_tiles[g] = (s_a, s_b)
                else:
                    s = spsum.tile([128, PCH], f32, tag="s", name="s")
                    nc.tensor.matmul(
                        s[:NLAST],
                        lhsT=_mm_ap(mk2_sb[0:64, g, :NLAST]),
                        rhs=_mm_ap(qk2_sb[0:64, ps:ps + PCH]),
                        start=True,
                        stop=True,
                        tile_position=(0, 0),
                    )
                    s_pool_tiles[g] = (s,)

            for pc in range(NPC):
                ps = pc * PCH
                acc = []
                for vt in range(4):
                    acc.append(accpsum.tile([128, PCH], f32, tag="acc", name="acc"))
                lacc = lpsum.tile([1, PCH], f32, tag="lacc", name="lacc")

                s_tiles = {}
                issue_group(ps, 0, s_tiles)
                for g in range(len(groups)):
                    if g + 1 < len(groups):
                        issue_group(ps, g + 1, s_tiles)
                    for k, nt in enumerate(groups[g]):
                        nsz = 128 if nt < NT - 1 else NLAST
                        s_cur = s_tiles[g][k]
                        e_sb = epool.tile([128, PCH], dt, tag="e", name="e")
                        nc.scalar.activation(
                            out=e_sb[:nsz],
                            in_=s_cur[:nsz],
                            func=mybir.ActivationFunctionType.Exp,
                            scale=0.125,  # 1/sqrt(CK)
                        )
                        first, last = nt == 0, nt == NT - 1
                        for vt in range(4):
                            nc.tensor.matmul(
                                acc[vt],
                                lhsT=_mm_ap(mvt_sb[:nsz, nt, vt * 128:(vt + 1) * 128]),
                                rhs=_mm_ap(e_sb[:nsz]),
                                start=first,
                                stop=last,
                            )
                        nc.tensor.matmul(
                            lacc,
                            lhsT=_mm_ap(ones_sb[:nsz]),
                            rhs=_mm_ap(e_sb[:nsz]),
                            start=first,
                            stop=last,
                        )
                    del s_tiles[g]

                for vt in range(4):
                    o_sb = opool.tile([128, PCH], f32, tag="o", name="o")
                    nc.vector.tensor_copy(out=o_sb, in_=acc[vt])
                    nc.sync.dma_start(
                        out=mem_d[vt * 128:(vt + 1) * 128, ps:ps + PCH], in_=o_sb
                    )
                ol_sb = olpool.tile([1, PCH], f32, tag="ol", name="ol")
                nc.vector.tensor_copy(out=ol_sb, in_=lacc)
                nc.sync.dma_start(out=l_d[:, ps:ps + PCH], in_=ol_sb)

    nc.compile()
    return nc


def _get_program():
    if "nc" not in _CACHE:
        _CACHE["nc"] = _build_program()
    return _CACHE["nc"]


def _make_in_maps(mk, mv, qk):
    npdt = _np_dtype()
    mkf = np.ascontiguousarray(mk.reshape(B, CK, N))
    mvf = np.ascontiguousarray(mv.reshape(B, CV, N))
    qkf = np.ascontiguousarray(qk.reshape(B, CK, P))
    in_maps = []
    for core in range(8):
        b, half = core // 2, core % 2
        n0, n1 = half * NHALF, (half + 1) * NHALF
        mk_c = mkf[b, :, n0:n1].astype(npdt)          # [64, 6480]
        # row-pair-packed mk: pair j = (tile 2j in parts 0:64, tile 2j+1 in
        # parts 64:128); leftover tile NT-1 (80 cols) in the last slot
        mk2 = np.zeros((128, NPAIR + 1, 128), dtype=npdt)
        paired = mk_c[:, :NPAIR * 256].reshape(CK, NPAIR, 2, 128)
        mk2[0:64, :NPAIR, :] = paired[:, :, 0, :]
        mk2[64:128, :NPAIR, :] = paired[:, :, 1, :]
        mk2[0:64, NPAIR, :NLAST] = mk_c[:, NPAIR * 256:]
        qk_c = qkf[b].astype(npdt)
        qk2 = np.concatenate([qk_c, qk_c], axis=0)    # duplicate into 64:128
        mvt = np.zeros((NPAD, CV), dtype=npdt)
        mvt[:NHALF] = mvf[b, :, n0:n1].T
        # partition-major tiles: [128, NT, CV], elem (p, t, v) = mvT[t*128+p, v]
        mvt_c = np.ascontiguousarray(mvt.reshape(NT, 128, CV).transpose(1, 0, 2))
        in_maps.append({"mk2": np.ascontiguousarray(mk2),
                        "qk2": np.ascontiguousarray(qk2),
                        "mvT": mvt_c})
    return in_maps


def _run(mk, mv, qk, qv, trace=False, **spmd_kwargs):
    nc = _get_program()
    in_maps = _make_in_maps(mk, mv, qk)
    res = run_bass_kernel_spmd(nc, in_maps, list(range(8)), trace=trace, **spmd_kwargs)
    out = np.empty((B, 2 * CV, P), dtype=np.float32)
    for b in range(B):
        m0, l0 = res.results[2 * b]["mem"], res.results[2 * b]["lsum"]
        m1, l1 = res.results[2 * b + 1]["mem"], res.results[2 * b + 1]["lsum"]
        out[b, :CV] = (m0 + m1) / (l0 + l1)
        out[b, CV:] = qv[b].reshape(CV, P)
    return out.reshape(B, 2 * CV, H, W), res


def kernel(mk, mv, qk, qv):
    out, _ = _run(np.asarray(mk), np.asarray(mv), np.asarray(qk), np.asarray(qv))
    return out


# revision 15
# speedup vs baseline: 1.3264x; 1.1969x over previous
"""Trainium2 Bass kernel for nn_MemoryReader.

Reference computation (per batch b):
    mi = mk.reshape(CK, N);  qi = qk.reshape(CK, P) / sqrt(CK)
    S  = mi.T @ qi                      # [N, P] affinity logits
    A  = softmax(S, axis=0)             # over memory axis N
    mem = mv.reshape(CV, N) @ A         # [CV, P]
    out = concat([mem, qv], axis=channel)

Sharding: 8 cores = (4 batches) x (2 halves of the memory axis N).
Each core computes, for its (b, half):
    E      = exp(S_half)                        # no max subtraction (logits ~ N(0,1))
    mem_un = mv_half @ E                        # [CV, P] unnormalized numerator
    lsum   = ones @ E                           # [1, P] denominator part
The host combines: mem = (mem_un_0 + mem_un_1) / (lsum_0 + lsum_1), then
concats qv (pure passthrough). No on-device collectives needed.

Device layout notes:
  - E is produced directly in [n(partition), p(free)] layout by computing
    S = mk_tile.T @ qk (lhsT = mk slice, K=CK=64 on partitions).
  - The second matmul contracts over n, so both operands need n on
    partitions: mv is pre-transposed ON THE HOST into [128, NT, CV]
    (partition-major tiles), making the device program transpose-free.
  - 1/sqrt(CK) is folded into the exp activation's free affine scale.
"""

import numpy as np
import ml_dtypes

import concourse.bass as bass
import concourse.tile as tile
from concourse import bacc, mybir
from concourse.bass_utils import run_bass_kernel_spmd

# Problem shape (hardcoded per contract)
B, CK, CV, T, H, W = 4, 64, 512, 8, 30, 54
N = T * H * W          # 12960 memory positions
P = H * W              # 1620 query positions
NHALF = N // 2         # 6480 per core
NT = (NHALF + 127) // 128   # 51 n-tiles (last has 80 rows)
NLAST = NHALF - (NT - 1) * 128  # 80
NPAD = NT * 128        # 6528
NPAIR = NT // 2        # 25 row-packed mm1 pairs (+1 solo leftover tile)
# p-axis chunking: chunks of 512 (one PSUM bank for mm1 out), each chunk
# further sliced into 128-wide pieces that serve as mm2 stationary weights
PCHUNKS = [512, 512, 512, 84]
PSLICES = [4, 4, 4, 1]      # 13 slices of (128,...,128,84) total
NSL = 13

# Matmul precision mode: "bf16" (1 cyc/col), "f32r" (fp32 data, ~1 cyc/col
# at free>=256), "f32" (4 cyc/col).
MM_MODE = "bf16"

_CACHE = {}


def _mm_dtype():
    return mybir.dt.bfloat16 if MM_MODE == "bf16" else mybir.dt.float32


def _np_dtype():
    return ml_dtypes.bfloat16 if MM_MODE == "bf16" else np.float32


def _mm_ap(ap):
    """Operand view handed to the tensor engine."""
    if MM_MODE == "f32r":
        return ap.bitcast(mybir.dt.float32r)
    return ap


def _build_program():
    dt = _mm_dtype()
    f32 = mybir.dt.float32
    # Bacc (not plain Bass): its compile() runs generate_event_semaphores,
    # which splits multi-wait sync_info onto EventSemaphore instructions
    # (TRN2 allows only one wait per regular instruction).
    nc = bacc.Bacc(None, target_bir_lowering=False, debug=False)

    # mk2: row-pair-packed mk. Pair j holds n-tile 2j in partitions 0:64 and
    # n-tile 2j+1 in partitions 64:128 (the PE contraction dim is CK=64, so
    # two mm1 matmuls run concurrently in disjoint row groups). qk2 is qk
    # duplicated into partitions 64:128 (the row-group-1 matmul streams its
    # rhs from those partitions).
    mk2_d = nc.declare_dram_parameter("mk2", [128, NPAIR + 1, 128], dt, isOutput=False)
    qk2_d = nc.declare_dram_parameter("qk2", [128, P], dt, isOutput=False)
    mvt_d = nc.declare_dram_parameter("mvT", [128, NT, CV], dt, isOutput=False)
    # outputs in transposed layout: memT[p, v]; lsum packed [row, slice] with
    # l[p] at row=p%128, slice=p//128
    mem_d = nc.declare_dram_parameter("memT", [P, CV], f32, isOutput=True)
    l_d = nc.declare_dram_parameter("lsum", [128, NSL], f32, isOutput=True)

    with tile.TileContext(nc) as tc:
        with (
            tc.tile_pool(name="singles", bufs=1) as singles,
            tc.tile_pool(name="epool", bufs=3) as epool,
            tc.tile_pool(name="opool", bufs=8) as opool,
            tc.tile_pool(name="olpool", bufs=2) as olpool,
            tc.tile_pool(name="spsum", bufs=3, space="PSUM") as spsum,
            tc.tile_pool(name="accpsum", bufs=4, space="PSUM") as accpsum,
            tc.tile_pool(name="lpsum", bufs=1, space="PSUM") as lpsum,
        ):
            ones_sb = singles.tile([128, 1], dt)
            nc.vector.memset(ones_sb, 1.0)
            qk2_sb = singles.tile([128, P], dt)
            nc.sync.dma_start(out=qk2_sb, in_=qk2_d[:])
            mk2_sb = singles.tile([128, NPAIR + 1, 128], dt)
            for g in range(0, NPAIR + 1, 7):
                g1 = min(g + 7, NPAIR + 1)
                nc.sync.dma_start(out=mk2_sb[:, g:g1, :], in_=mk2_d[:, g:g1, :])
            mvt_sb = singles.tile([128, NT, CV], dt)
            # split the load along the n-tile axis: each weight-tile read then
            # depends on exactly one DMA (avoids multi-sem wait explosion),
            # and the chunks land in consumption order across parallel queues
            NTG = 3
            for g in range(0, NT, NTG):
                g1 = min(g + NTG, NT)
                nc.sync.dma_start(
                    out=mvt_sb[:, g:g1, :],
                    in_=mvt_d[:, g:g1, :],
                )

            # Warm-up: ~128 tiny matmuls depending only on the ones memset.
            # They run while the input DMAs land, filling the initial PE idle
            # gap and releasing the HAM clock throttle (~3.4us of activity)
            # before the real work starts.
            warm = lpsum.tile([1, 1], f32, tag="lacc", name="warm")
            for _ in range(128):
                nc.tensor.matmul(
                    warm[0:1, 0:1],
                    lhsT=_mm_ap(ones_sb),
                    rhs=_mm_ap(ones_sb),
                    start=True,
                    stop=True,
                )

            # n-tile groups: 25 row-packed pairs + the 80-row leftover tile
            groups = [(2 * j, 2 * j + 1) for j in range(NPAIR)] + [(NT - 1,)]

            def issue_group(ps, w, g, s_pool_tiles):
                if len(groups[g]) == 2:
                    s_a = spsum.tile([128, 512], f32, tag="s", name="s")
                    s_b = spsum.tile([128, 512], f32, tag="s", name="s")
                    nc.tensor.matmul(
                        s_a[:, :w],
                        lhsT=_mm_ap(mk2_sb[0:64, g, :]),
                        rhs=_mm_ap(qk2_sb[0:64, ps:ps + w]),
                        start=True,
                        stop=True,
                        tile_position=(0, 0),
                    )
                    nc.tensor.matmul(
                        s_b[:, :w],
                        lhsT=_mm_ap(mk2_sb[64:128, g, :]),
                        rhs=_mm_ap(qk2_sb[64:128, ps:ps + w]),
                        start=True,
                        stop=True,
                        tile_position=(64, 0),
                    )
                    s_pool_tiles[g] = (s_a, s_b)
                else:
                    s = spsum.tile([128, 512], f32, tag="s", name="s")
                    nc.tensor.matmul(
                        s[:NLAST, :w],
                        lhsT=_mm_ap(mk2_sb[0:64, g, :NLAST]),
                        rhs=_mm_ap(qk2_sb[0:64, ps:ps + w]),
                        start=True,
                        stop=True,
                        tile_position=(0, 0),
                    )
                    s_pool_tiles[g] = (s,)

            ps = 0
            sl0 = 0  # global slice index of this chunk's first slice
            for ci, (w, nsl) in enumerate(zip(PCHUNKS, PSLICES)):
                acc = []
                for sl in range(nsl):
                    acc.append(accpsum.tile([128, CV], f32, tag="acc", name="acc"))
                # all slices of this chunk accumulate l into ONE psum bank:
                # only the first matmul of the bank uses start=True (whole-bank
                # has_written clear); later slices' first matmuls rely on the
                # per-element overwrite-when-bit-unset semantics.
                lacc = lpsum.tile([128, NSL], f32, tag="lacc", name="lacc")

                s_tiles = {}
                issue_group(ps, w, 0, s_tiles)
                for g in range(len(groups)):
                    if g + 1 < len(groups):
                        issue_group(ps, w, g + 1, s_tiles)
                    for k, nt in enumerate(groups[g]):
                        nsz = 128 if nt < NT - 1 else NLAST
                        s_cur = s_tiles[g][k]
                        e_sb = epool.tile([128, 512], dt, tag="e", name="e")
                        nc.scalar.activation(
                            out=e_sb[:nsz, :w],
                            in_=s_cur[:nsz, :w],
                            func=mybir.ActivationFunctionType.Exp,
                            scale=0.125,  # 1/sqrt(CK)
                        )
                        first, last = nt == 0, nt == NT - 1
                        for sl in range(nsl):
                            pw = min(128, w - sl * 128)
                            el = e_sb[:nsz, sl * 128:sl * 128 + pw]
                            nc.tensor.matmul(
                                acc[sl][:pw],
                                lhsT=_mm_ap(el),
                                rhs=_mm_ap(mvt_sb[:nsz, nt, :]),
                                start=first,
                                stop=last,
                            )
                            # denominator: same stationary weights, ones rhs
                            nc.tensor.matmul(
                                lacc[:pw, sl0 + sl:sl0 + sl + 1],
                                lhsT=_mm_ap(el),
                                rhs=_mm_ap(ones_sb[:nsz]),
                                start=first and sl == 0,
                                stop=last,
                                skip_group_check=True,
                            )
                    del s_tiles[g]

                for sl in range(nsl):
                    pw = min(128, w - sl * 128)
                    o_sb = opool.tile([128, CV], f32, tag="o", name="o")
                    nc.vector.tensor_copy(out=o_sb[:pw], in_=acc[sl][:pw])
                    p0 = ps + sl * 128
                    nc.sync.dma_start(out=mem_d[p0:p0 + pw, :], in_=o_sb[:pw])
                ol_sb = olpool.tile([128, NSL], f32, tag="ol", name="ol")
                nc.vector.tensor_copy(
                    out=ol_sb[:, sl0:sl0 + nsl], in_=lacc[:, sl0:sl0 + nsl]
                )
                nc.sync.dma_start(
                    out=l_d[:, sl0:sl0 + nsl], in_=ol_sb[:, sl0:sl0 + nsl]
                )
                ps += w
                sl0 += nsl

    nc.compile()
    return nc


def _get_program():
    if "nc" not in _CACHE:
        _CACHE["nc"] = _build_program()
    return _CACHE["nc"]


def _make_in_maps(mk, mv, qk):
    npdt = _np_dtype()
    mkf = np.ascontiguousarray(mk.reshape(B, CK, N))
    mvf = np.ascontiguousarray(mv.reshape(B, CV, N))
    qkf = np.ascontiguousarray(qk.reshape(B, CK, P))
    in_maps = []
    for core in range(8):
        b, half = core // 2, core % 2
        n0, n1 = half * NHALF, (half + 1) * NHALF
        mk_c = mkf[b, :, n0:n1].astype(npdt)          # [64, 6480]
        # row-pair-packed mk: pair j = (tile 2j in parts 0:64, tile 2j+1 in
        # parts 64:128); leftover tile NT-1 (80 cols) in the last slot
        mk2 = np.zeros((128, NPAIR + 1, 128), dtype=npdt)
        paired = mk_c[:, :NPAIR * 256].reshape(CK, NPAIR, 2, 128)
        mk2[0:64, :NPAIR, :] = paired[:, :, 0, :]
        mk2[64:128, :NPAIR, :] = paired[:, :, 1, :]
        mk2[0:64, NPAIR, :NLAST] = mk_c[:, NPAIR * 256:]
        qk_c = qkf[b].astype(npdt)
        qk2 = np.concatenate([qk_c, qk_c], axis=0)    # duplicate into 64:128
        mvt = np.zeros((NPAD, CV), dtype=npdt)
        mvt[:NHALF] = mvf[b, :, n0:n1].T
        # partition-major tiles: [128, NT, CV], elem (p, t, v) = mvT[t*128+p, v]
        mvt_c = np.ascontiguousarray(mvt.reshape(NT, 128, CV).transpose(1, 0, 2))
        in_maps.append({"mk2": np.ascontiguousarray(mk2),
                        "qk2": np.ascontiguousarray(qk2),
                        "mvT": mvt_c})
    return in_maps


def _run(mk, mv, qk, qv, trace=False, **spmd_kwargs):
    nc = _get_program()
    in_maps = _make_in_maps(mk, mv, qk)
    res = run_bass_kernel_spmd(nc, in_maps, list(range(8)), trace=trace, **spmd_kwargs)
    out = np.empty((B, 2 * CV, P), dtype=np.float32)
    for b in range(B):
        m0, l0 = res.results[2 * b]["memT"], res.results[2 * b]["lsum"]
        m1, l1 = res.results[2 * b + 1]["memT"], res.results[2 * b + 1]["lsum"]
        # memT is [P, CV]; lsum is [128, NSL] with l[p] at [p % 128, p // 128]
        lv = (l0 + l1).T.reshape(-1)[:P]            # [P]
        out[b, :CV] = ((m0 + m1) / lv[:, None]).T
        out[b, CV:] = qv[b].reshape(CV, P)
    return out.reshape(B, 2 * CV, H, W), res


def kernel(mk, mv, qk, qv):
    out, _ = _run(np.asarray(mk), np.asarray(mv), np.asarray(qk), np.asarray(qv))
    return out
